# revision 29
# baseline (speedup 1.0000x reference)
"""Pointer-network (enc LSTM -> dec LSTM + attention) Trainium2 Bass kernel.

Sharding: pure data-parallel over batch B=256 across 8 NeuronCores (32/core).
Everything SBUF-resident per core; sequential scan over L stays on-core.

Per-core layouts (p = partition dim):
  hT/cT       [128 hsub, 4 hc, 32 b]          state, h-dim = hc*128+hsub
  E_sb        [128 hsub, 128 l, 128 (hc*32+b)] = enc_out @ w1.T (bf16)
  T_sb        same layout, tanh(E + q) per decode step
  H_all       [128 hsub, 4 hc, 32 b, 128 l]    encoder outputs
  E0T         [128 l, 32 b, 4 hc, 128 hsub]    enc_out transposed (context lhsT)
  gates       [128 gsub, 16 gc, 32 b]          PSUM, gate-dim = gc*128+gsub

Key scheduling: gate mms that depend only on h/bias fire BEFORE the big
tanh window (PE fills the ACT-bound phase); Wi@ctx mms append to the same
PSUM accumulation group after ctx (one start/stop per bank -- start=True
zero-clears the whole bank). Softmax skips max-subtraction (scores are
small; exp/sum/log all fp32-safe). LSTM pointwise = ONE tanh over all 16
gate chunks (0.5 sigmoid scale folded into i,f,o weight rows on host) +
cheap DVE fixups. Inputs preloaded to SBUF once (no per-step input DMA);
per-step raw scores stream to a DRAM stash off the critical path and the
deferred log-softmax correction runs at the end (Ln loaded once). Graded
tanh blocks (16/32/32/32/16 l) pipeline DVE adds + ACT tanh + PE score
mms; score extraction happens per block under the tanh window; ctx mms
run hc-outer with per-hc casts so Wi can chase them. 8 steps inline per
hardware-loop iteration amortize the per-iteration ACT_TABLE_LOAD (a
dummy tiny ACT absorbs it off the critical path) and loop-boundary
drains.
"""

import os
import sys

import numpy as np

for _p in ("/opt/trn_rl_repo", os.environ.get("TRN_RL_REPO", "")):
    if _p and _p not in sys.path and os.path.isdir(_p):
        sys.path.insert(0, _p)

import ml_dtypes

bf16 = ml_dtypes.bfloat16

B, L, H = 256, 128, 512
NCORES = 8
BL = B // NCORES  # 32
HC = H // 128     # 4
GC = 4 * H // 128  # 16

_cache = {}


def _build_nc(unroll=4):
    import concourse.bass as bass
    import concourse.bacc as bacc
    import concourse.tile as tile
    from concourse import mybir
    from concourse.masks import make_identity

    AFT = mybir.ActivationFunctionType
    ALU = mybir.AluOpType
    f32 = mybir.dt.float32
    b16 = mybir.dt.bfloat16

    nc = bacc.Bacc("TRN2", target_bir_lowering=False, debug=False)

    xa_d = nc.dram_tensor("xa", [2, L * BL], b16, kind="ExternalInput").ap()
    dt_d = nc.dram_tensor("dt", [2, L * BL], b16, kind="ExternalInput").ap()
    ewb_d = nc.dram_tensor("ewb", [2, 4 * H], b16, kind="ExternalInput").ap()
    dwb_d = nc.dram_tensor("dwb", [2, 4 * H], b16, kind="ExternalInput").ap()
    ewhT_d = nc.dram_tensor("ewhT", [128, HC, 4 * H], b16, kind="ExternalInput").ap()
    dwhT_d = nc.dram_tensor("dwhT", [128, HC, 4 * H], b16, kind="ExternalInput").ap()
    dwiT_d = nc.dram_tensor("dwiT", [128, HC, 4 * H], b16, kind="ExternalInput").ap()
    w1T_d = nc.dram_tensor("w1T", [128, HC, H], b16, kind="ExternalInput").ap()
    w2T_d = nc.dram_tensor("w2T", [128, HC, H], b16, kind="ExternalInput").ap()
    v4_d = nc.dram_tensor("v4", [128, HC], b16, kind="ExternalInput").ap()
    outp_d = nc.dram_tensor("outp", [BL, L, L], f32, kind="ExternalOutput").ap()
    sstore_d = nc.dram_tensor("sstore", [L, BL, L], f32).ap()


    with tile.TileContext(nc) as tc, tc.tile_pool(name="perm", bufs=1) as perm:
        E_sb = perm.tile([128, L, 128], b16)
        E0T = perm.tile([128, BL, HC, 128], b16)
        dwhT = perm.tile([128, HC, 4 * H], b16)
        dwiT = perm.tile([128, HC, 4 * H], b16)
        w1T = perm.tile([128, HC, H], b16)
        w2T = perm.tile([128, HC, H], b16)
        dwb = perm.tile([2, 4 * H], b16)
        v4 = perm.tile([128, HC], b16)
        eye = perm.tile([128, 128], f32)
        eye16 = perm.tile([128, 128], b16)
        hT = perm.tile([128, HC, BL], b16)
        cT = perm.tile([128, HC, BL], b16)
        s_all = perm.tile([BL, L], f32)
        xa_sb = perm.tile([2, L * BL], b16)
        dt_sb = perm.tile([2, L * BL], b16)

        nc.sync.dma_start(dwhT, dwhT_d)
        nc.sync.dma_start(dwiT, dwiT_d)
        nc.sync.dma_start(w1T, w1T_d)
        nc.sync.dma_start(w2T, w2T_d)
        nc.sync.dma_start(dwb, dwb_d)
        nc.sync.dma_start(v4, v4_d)
        nc.sync.dma_start(xa_sb, xa_d)
        nc.sync.dma_start(dt_sb, dt_d)
        make_identity(nc, eye)
        nc.vector.tensor_copy(eye16, eye)
        nc.vector.memset(hT, 0.0)
        nc.vector.memset(cT, 0.0)

        def lstm_pointwise(work, g_ps, write_hall=None, iv=None):
            """gates PSUM [128, 16, 32] -> update hT, cT.

            Host folded 0.5 into i,f,o weight rows, so sigmoid(z) =
            (1 + tanh(z')) / 2 with z' the PSUM value; g uses tanh(z')
            directly (its rows unscaled). One tanh over all 16 chunks.
            """
            t_all = work.tile([128, GC, BL], b16, tag="tall")
            nc.scalar.activation(t_all, g_ps, AFT.Tanh, scale=1.0)
            sgi = work.tile([128, HC, BL], b16, tag="sgi")
            sgf = work.tile([128, HC, BL], b16, tag="sgf")
            sgo = work.tile([128, HC, BL], b16, tag="sgo")
            for sg, lo in ((sgf, 4), (sgi, 0)):
                nc.vector.tensor_scalar(out=sg, in0=t_all[:, lo:lo + 4, :],
                                        scalar1=1.0, scalar2=0.5,
                                        op0=ALU.add, op1=ALU.mult)
            u = work.tile([128, HC, BL], b16, tag="u")
            v = work.tile([128, HC, BL], b16, tag="v")
            nc.vector.tensor_mul(v, sgf, cT)
            nc.vector.tensor_mul(u, sgi, t_all[:, 8:12, :])
            nc.vector.tensor_add(cT, u, v)
            thc = work.tile([128, HC, BL], b16, tag="thc")
            nc.scalar.activation(thc, cT, AFT.Tanh, scale=1.0)
            nc.vector.tensor_scalar(out=sgo, in0=t_all[:, 12:16, :],
                                    scalar1=1.0, scalar2=0.5,
                                    op0=ALU.add, op1=ALU.mult)
            nc.vector.tensor_mul(hT, sgo, thc)
            if write_hall is not None:
                dst = write_hall[:, :, :, bass.ds(iv, 1)]
                nc.vector.tensor_copy(dst, hT.unsqueeze(-1))

        # ---------------- encoder ----------------
        with tc.tile_pool(name="encp", bufs=1) as encp, \
             tc.tile_pool(name="encw", bufs=3) as encw, \
             tc.tile_pool(name="encr", bufs=4) as encr, \
             tc.tile_pool(name="psg", bufs=2, space="PSUM") as psg, \
             tc.tile_pool(name="pse", bufs=2, space="PSUM") as pse:
            ewhT = encp.tile([128, HC, 4 * H], b16)
            ewb = encp.tile([2, 4 * H], b16)
            H_all = encp.tile([128, HC, BL, L], b16)
            nc.sync.dma_start(ewhT, ewhT_d)
            nc.sync.dma_start(ewb, ewb_d)

            def emit_eps(iv):
                # E_sb[iv] = w1 @ h_iv -- emitted one body later (hT still
                # holds h_iv there) so these mms fill the pointwise window
                e_ps = pse.tile([128, HC, BL], mybir.dt.float32, tag="eps")
                for pc in range(HC):
                    for kc in range(HC):
                        nc.tensor.matmul(
                            e_ps[:, pc, :],
                            lhsT=w1T[:, kc, pc * 128:(pc + 1) * 128],
                            rhs=hT[:, kc, :], start=(kc == 0), stop=(kc == 3))
                dst = E_sb[:, bass.ds(iv, 1), :]
                nc.vector.tensor_copy(
                    dst, e_ps.rearrange("p a b -> p (a b)").unsqueeze(1))

            def enc_body(iv, k):
                if k == 0:
                    dum = encr.tile([1, 1], b16, tag="dum")
                    nc.scalar.activation(dum, eye16[0:1, 0:1], AFT.Tanh,
                                         scale=1.0)
                xa_t = encr.tile([2, BL], b16, tag="xa")
                nc.vector.tensor_copy(xa_t, xa_sb[:, bass.ds(iv * BL, BL)])
                g_ps = psg.tile([128, GC, BL], mybir.dt.float32, tag="gps")
                for gc in range(GC):
                    nc.tensor.matmul(
                        g_ps[:, gc, :], lhsT=ewb[:, gc * 128:(gc + 1) * 128],
                        rhs=xa_t, start=(gc == 0), stop=False)
                for gc in range(GC):
                    for kc in range(HC):
                        nc.tensor.matmul(
                            g_ps[:, gc, :],
                            lhsT=ewhT[:, kc, gc * 128:(gc + 1) * 128],
                            rhs=hT[:, kc, :], start=False,
                            stop=(gc == GC - 1 and kc == HC - 1))
                if k > 0:
                    emit_eps(iv - 1)
                lstm_pointwise(encw, g_ps, write_hall=H_all, iv=iv)

            with tc.For_i(0, L // 8, 1) as ordv:
                for k in range(8):
                    enc_body(ordv * 8 + k, k)
                emit_eps(ordv * 8 + 7)

            # H_all [hsub, hc, b, l] -> E0T [l, b, hc, hsub] via 128 PE transposes
            with tc.tile_pool(name="pst", bufs=2, space="PSUM") as pst, \
                 tc.tile_pool(name="trw", bufs=3) as trw:
                for hc in range(HC):
                    for b in range(BL):
                        tr_ps = pst.tile([128, 128], b16, tag="tr")
                        nc.tensor.transpose(tr_ps, H_all[:, hc, b, :], eye16)
                        if (hc * BL + b) % 2 == 0:
                            nc.vector.tensor_copy(E0T[:, b, hc, :], tr_ps)
                        else:
                            nc.scalar.copy(E0T[:, b, hc, :], tr_ps)

        # ---------------- decoder ----------------
        with tc.tile_pool(name="decp", bufs=1) as decp, \
             tc.tile_pool(name="decw", bufs=3) as decw, \
             tc.tile_pool(name="decx", bufs=2) as decx, \
             tc.tile_pool(name="decr", bufs=4) as decr, \
             tc.tile_pool(name="psq", bufs=1, space="PSUM") as psq, \
             tc.tile_pool(name="psz", bufs=2, space="PSUM") as psz, \
             tc.tile_pool(name="psa", bufs=1, space="PSUM") as psa, \
             tc.tile_pool(name="psc", bufs=1, space="PSUM") as psc, \
             tc.tile_pool(name="psg2", bufs=2, space="PSUM") as psg2:
            T_sb = decp.tile([128, L, 128], b16)

            def dec_body(iv, k, ordv):
                if k == 0:
                    # tiny dummy: absorbs the per-iteration ACT_TABLE_LOAD
                    # while PE/DVE run the front phase (else it delays tanh c0)
                    dum = decr.tile([1, 1], b16, tag="dum")
                    nc.scalar.activation(dum, eye16[0:1, 0:1], AFT.Tanh,
                                         scale=1.0)
                # q.T [hsub, hc, b] -- first on PE after h so the tanh
                # window opens as early as possible
                q_ps = psq.tile([128, HC, BL], mybir.dt.float32, tag="qps")
                for pc in range(HC):
                    for kc in range(HC):
                        nc.tensor.matmul(
                            q_ps[:, pc, :],
                            lhsT=w2T[:, kc, pc * 128:(pc + 1) * 128],
                            rhs=hT[:, kc, :], start=(kc == 0), stop=(kc == 3))
                qT = decw.tile([128, HC, BL], b16, tag="qT")
                nc.vector.tensor_copy(qT, q_ps)
                qflat = qT.rearrange("p a b -> p (a b)")
                # gates: bias + Wh@h mms next -- they run on PE during the
                # ACT-bound tanh window. Wi@ctx appends after ctx below.
                dt_t = decr.tile([2, BL], b16, tag="dt")
                nc.vector.tensor_copy(dt_t, dt_sb[:, bass.ds(iv * BL, BL)])
                g_ps = psg2.tile([128, GC, BL], mybir.dt.float32, tag="gps2")
                for gc in range(GC):
                    nc.tensor.matmul(
                        g_ps[:, gc, :], lhsT=dwb[:, gc * 128:(gc + 1) * 128],
                        rhs=dt_t, start=(gc == 0), stop=False)
                for gc in range(GC):
                    for kc in range(HC):
                        nc.tensor.matmul(
                            g_ps[:, gc, :],
                            lhsT=dwhT[:, kc, gc * 128:(gc + 1) * 128],
                            rhs=hT[:, kc, :], start=False, stop=False)
                # X = E + q (broadcast over l), T = tanh(X); leading
                # blocks smaller so the first tanh can start sooner
                for lo, ln_ in ((0, 16), (16, 32), (48, 32), (80, 40),
                                (120, 8)):
                    X_big = decx.tile([128, 40, 128], b16, tag="X")
                    X_blk = X_big[:, 0:ln_, :]
                    q_b = bass.AP(tensor=qflat.tensor, offset=qflat.offset,
                                  ap=[qflat.ap[0], [0, ln_], qflat.ap[1]])
                    nc.vector.tensor_add(
                        X_blk, E_sb[:, lo:lo + ln_, :], q_b)
                    nc.scalar.activation(
                        T_sb[:, lo:lo + ln_, :], X_blk, AFT.Tanh, scale=1.0)
                # scores: Z[l] = T_l.T @ v4, extracted per l-block so the
                # DVE reduction hides under the tanh window
                Z_ps = psz.tile([128, L, HC], mybir.dt.float32, tag="zps")
                S_sb = decw.tile([BL, L], mybir.dt.float32, tag="S")
                for lo, ln_ in ((0, 16), (16, 32), (48, 32), (80, 40),
                                (120, 8)):
                    for l in range(lo, lo + ln_):
                        nc.tensor.matmul(Z_ps[:, l, :], lhsT=T_sb[:, l, :],
                                         rhs=v4, start=True, stop=True)
                    sl = slice(lo, lo + ln_)
                    nc.vector.tensor_copy(S_sb[:, sl], Z_ps[0:32, sl, 0])
                    nc.vector.tensor_add(S_sb[:, sl], S_sb[:, sl],
                                         Z_ps[32:64, sl, 1])
                    nc.vector.tensor_add(S_sb[:, sl], S_sb[:, sl],
                                         Z_ps[64:96, sl, 2])
                    nc.vector.tensor_add(S_sb[:, sl], S_sb[:, sl],
                                         Z_ps[96:128, sl, 3])
                # softmax without max-subtraction (|S| <~ 20, fp32-safe)
                e_sb = decw.tile([BL, L], mybir.dt.float32, tag="e")
                s_t = decw.tile([BL, 1], mybir.dt.float32, tag="s")
                nc.scalar.activation(e_sb, S_sb, AFT.Exp, scale=1.0)
                nc.vector.tensor_reduce(out=s_t, in_=e_sb,
                                        axis=mybir.AxisListType.X, op=ALU.add)
                # stash raw scores (off critical path) + per-step sums
                nc.sync.dma_start(sstore_d[bass.ds(iv, 1), :, :], S_sb)
                nc.vector.tensor_copy(s_all[:, bass.ds(iv, 1)], s_t)
                r = decw.tile([BL, 1], mybir.dt.float32, tag="r")
                nc.vector.reciprocal(r, s_t)
                a_sb = decw.tile([BL, L], b16, tag="a")
                nc.vector.tensor_scalar_mul(a_sb, e_sb, r)
                # context
                aT_ps = psa.tile([128, BL], b16, tag="aT")
                nc.tensor.transpose(aT_ps, a_sb, eye16[0:BL, 0:BL])
                aT = decw.tile([128, BL], b16, tag="aTs")
                nc.vector.tensor_copy(aT, aT_ps)
                ctx_ps = psc.tile([128, HC, BL], mybir.dt.float32, tag="cps")
                xT = decw.tile([128, HC, BL], b16, tag="xT")
                for hc in range(HC):
                    for b in range(BL):
                        nc.tensor.matmul(ctx_ps[:, hc, b:b + 1],
                                         lhsT=E0T[:, b, hc, :],
                                         rhs=aT[:, b:b + 1],
                                         start=True, stop=True)
                    # per-hc cast overlaps the next hc's ctx matmuls
                    nc.vector.tensor_copy(xT[:, hc, :], ctx_ps[:, hc, :])
                # gates: Wi@ctx completes the accumulation group (kc-outer so
                # each kc chunk only needs its own xT slice)
                for kc in range(HC):
                    for gc in range(GC):
                        nc.tensor.matmul(
                            g_ps[:, gc, :],
                            lhsT=dwiT[:, kc, gc * 128:(gc + 1) * 128],
                            rhs=xT[:, kc, :], start=False,
                            stop=(kc == HC - 1 and gc == GC - 1))
                lstm_pointwise(decw, g_ps)

            with tc.For_i(0, L // 8, 1) as ordv:
                for k in range(8):
                    dec_body(ordv * 8 + k, k, ordv)

        # ---------------- deferred log-softmax ----------------
        with tc.tile_pool(name="post", bufs=8) as post, \
             tc.tile_pool(name="postc", bufs=1) as postc:
            lns = postc.tile([BL, L], mybir.dt.float32)
            nc.scalar.activation(lns, s_all, AFT.Ln, scale=1.0)
            for t in range(L):
                S_t = post.tile([BL, L], mybir.dt.float32, tag="St")
                nc.scalar.dma_start(S_t, sstore_d[t, :, :])
                o_t = post.tile([BL, L], mybir.dt.float32, tag="ot")
                nc.vector.tensor_scalar(out=o_t, in0=S_t,
                                        scalar1=lns[:, t:t + 1], scalar2=None,
                                        op0=ALU.subtract)
                (nc.sync if t % 2 == 0 else nc.gpsimd).dma_start(
                    outp_d[:, t, :], o_t)

    nc.finalize()
    return nc


def _prep_weights(enc_Wi, enc_Wh, enc_b, dec_Wi, dec_Wh, dec_b, w1, w2, vt):
    """Host-side weight repack (shared across cores).

    0.5 folded into i,f,o gate rows so sigmoid(z) = (1+tanh(z'))/2 on-device.
    """
    f = np.float32
    sc = np.ones(4 * H, f)
    sc[0:H] = 0.5          # i
    sc[H:2 * H] = 0.5      # f
    sc[3 * H:4 * H] = 0.5  # o

    def chunkT(W):  # [4H, H] -> [128, HC, 4H]: out[p, kc, g] = W[g, kc*128+p]
        Wt = np.ascontiguousarray(W.astype(f).T)          # [H, 4H]
        return Wt.reshape(HC, 128, 4 * H).transpose(1, 0, 2).astype(bf16)

    def chunkT_sq(W):  # [H, H] -> [128, HC, H]
        Wt = np.ascontiguousarray(W.astype(f).T)
        return Wt.reshape(HC, 128, H).transpose(1, 0, 2).astype(bf16)

    ewb = np.stack([enc_Wi.astype(f)[:, 0] * sc, enc_b.astype(f) * sc]).astype(bf16)
    dwb = np.stack([dec_Wi.astype(f)[:, H] * sc, dec_b.astype(f) * sc]).astype(bf16)
    return {
        "ewb": ewb, "dwb": dwb,
        "ewhT": chunkT(enc_Wh * sc[:, None]),
        "dwhT": chunkT(dec_Wh * sc[:, None]),
        "dwiT": chunkT(dec_Wi[:, :H] * sc[:, None]),
        "w1T": chunkT_sq(w1), "w2T": chunkT_sq(w2),
        "v4": vt.astype(f)[0].reshape(HC, 128).T.astype(bf16).copy(),
    }


def kernel(xs, x_lens, argsort_xs, enc_Wi, enc_Wh, enc_b,
           dec_Wi, dec_Wh, dec_b, w1, w2, vt):
    from concourse.bass_utils import run_bass_kernel_spmd

    if "nc" not in _cache:
        _cache["nc"] = _build_nc()
    nc = _cache["nc"]

    wmap = _prep_weights(enc_Wi, enc_Wh, enc_b, dec_Wi, dec_Wh, dec_b,
                         w1, w2, vt)
    xs_f = xs.astype(np.float32)
    D = np.concatenate(
        [np.zeros((B, 1), np.float32),
         np.take_along_axis(xs_f, argsort_xs[:, :-1].astype(np.int64), axis=1)],
        axis=1)  # [B, L] teacher-forced decoder inputs

    in_maps = []
    for c in range(NCORES):
        sl = slice(c * BL, (c + 1) * BL)
        xa = np.empty((2, L * BL), np.float32)
        xa[0] = xs_f[sl].T.reshape(-1)       # xa[0, l*BL+b] = xs[b, l]
        xa[1] = 1.0
        dt = np.empty((2, L * BL), np.float32)
        dt[0] = D[sl].T.reshape(-1)
        dt[1] = 1.0
        m = dict(wmap)
        m["xa"] = xa.astype(bf16)
        m["dt"] = dt.astype(bf16)
        in_maps.append(m)

    _cache["in_maps"] = in_maps
    res = run_bass_kernel_spmd(nc, in_maps, core_ids=list(range(NCORES)))
    out = np.concatenate([res.results[c]["outp"] for c in range(NCORES)], axis=0)
    return np.ascontiguousarray(out.astype(np.float32))


# revision 31
# speedup vs baseline: 1.0206x; 1.0206x over previous
"""Pointer-network (enc LSTM -> dec LSTM + attention) Trainium2 Bass kernel.

Sharding: pure data-parallel over batch B=256 across 8 NeuronCores (32/core).
Everything SBUF-resident per core; sequential scan over L stays on-core.

Per-core layouts (p = partition dim):
  hT/cT       [128 hsub, 4 hc, 32 b]          state, h-dim = hc*128+hsub
  E_sb        [128 hsub, 128 l, 128 (hc*32+b)] = enc_out @ w1.T (bf16)
  T_sb        same layout, tanh(E + q) per decode step
  H_all       [128 hsub, 4 hc, 32 b, 128 l]    encoder outputs
  E0T         [128 l, 32 b, 4 hc, 128 hsub]    enc_out transposed (context lhsT)
  gates       [128 gsub, 16 gc, 32 b]          PSUM, gate-dim = gc*128+gsub

Key scheduling: gate mms that depend only on h/bias fire BEFORE the big
tanh window (PE fills the ACT-bound phase); Wi@ctx mms append to the same
PSUM accumulation group after ctx (one start/stop per bank -- start=True
zero-clears the whole bank). Softmax skips max-subtraction (scores are
small; exp/sum/log all fp32-safe). LSTM pointwise = ONE tanh over all 16
gate chunks (0.5 sigmoid scale folded into i,f,o weight rows on host) +
cheap DVE fixups. Inputs preloaded to SBUF once (no per-step input DMA);
per-step raw scores stream to a DRAM stash off the critical path and the
deferred log-softmax correction runs at the end (Ln loaded once). Graded
tanh blocks (16/32/32/32/16 l) pipeline DVE adds + ACT tanh + PE score
mms; score extraction happens per block under the tanh window; ctx mms
run hc-outer with per-hc casts so Wi can chase them. 8 steps inline per
hardware-loop iteration amortize the per-iteration ACT_TABLE_LOAD (a
dummy tiny ACT absorbs it off the critical path) and loop-boundary
drains.
"""

import os
import sys

import numpy as np

for _p in ("/opt/trn_rl_repo", os.environ.get("TRN_RL_REPO", "")):
    if _p and _p not in sys.path and os.path.isdir(_p):
        sys.path.insert(0, _p)

import ml_dtypes

bf16 = ml_dtypes.bfloat16

B, L, H = 256, 128, 512
NCORES = 8
BL = B // NCORES  # 32
HC = H // 128     # 4
GC = 4 * H // 128  # 16

_cache = {}


def _build_nc(unroll=4):
    import concourse.bass as bass
    import concourse.bacc as bacc
    import concourse.tile as tile
    from concourse import mybir
    from concourse.masks import make_identity

    AFT = mybir.ActivationFunctionType
    ALU = mybir.AluOpType
    f32 = mybir.dt.float32
    b16 = mybir.dt.bfloat16

    nc = bacc.Bacc("TRN2", target_bir_lowering=False, debug=False)

    xa_d = nc.dram_tensor("xa", [2, L * BL], b16, kind="ExternalInput").ap()
    dt_d = nc.dram_tensor("dt", [2, L * BL], b16, kind="ExternalInput").ap()
    ewb_d = nc.dram_tensor("ewb", [2, 4 * H], b16, kind="ExternalInput").ap()
    dwb_d = nc.dram_tensor("dwb", [2, 4 * H], b16, kind="ExternalInput").ap()
    ewhT_d = nc.dram_tensor("ewhT", [128, HC, 4 * H], b16, kind="ExternalInput").ap()
    dwhT_d = nc.dram_tensor("dwhT", [128, HC, 4 * H], b16, kind="ExternalInput").ap()
    dwiT_d = nc.dram_tensor("dwiT", [128, HC, 4 * H], b16, kind="ExternalInput").ap()
    w1T_d = nc.dram_tensor("w1T", [128, HC, H], b16, kind="ExternalInput").ap()
    w2T_d = nc.dram_tensor("w2T", [128, HC, H], b16, kind="ExternalInput").ap()
    v4_d = nc.dram_tensor("v4", [128, HC], b16, kind="ExternalInput").ap()
    outp_d = nc.dram_tensor("outp", [BL, L, L], f32, kind="ExternalOutput").ap()
    sstore_d = nc.dram_tensor("sstore", [L, BL, L], f32).ap()


    with tile.TileContext(nc) as tc, tc.tile_pool(name="perm", bufs=1) as perm:
        E_sb = perm.tile([128, L, 128], b16)
        E0T = perm.tile([128, BL, HC, 128], b16)
        dwhT = perm.tile([128, HC, 4 * H], b16)
        dwiT = perm.tile([128, HC, 4 * H], b16)
        w1T = perm.tile([128, HC, H], b16)
        w2T = perm.tile([128, HC, H], b16)
        dwb = perm.tile([2, 4 * H], b16)
        v4 = perm.tile([128, HC], b16)
        eye = perm.tile([128, 128], f32)
        eye16 = perm.tile([128, 128], b16)
        hT = perm.tile([128, HC, BL], b16)
        cT = perm.tile([128, HC, BL], b16)
        s_all = perm.tile([BL, L], f32)
        xa_sb = perm.tile([2, L * BL], b16)
        dt_sb = perm.tile([2, L * BL], b16)

        nc.sync.dma_start(dwhT, dwhT_d)
        nc.sync.dma_start(dwiT, dwiT_d)
        nc.sync.dma_start(w1T, w1T_d)
        nc.sync.dma_start(w2T, w2T_d)
        nc.sync.dma_start(dwb, dwb_d)
        nc.sync.dma_start(v4, v4_d)
        nc.sync.dma_start(xa_sb, xa_d)
        nc.sync.dma_start(dt_sb, dt_d)
        make_identity(nc, eye)
        nc.vector.tensor_copy(eye16, eye)
        nc.vector.memset(hT, 0.0)
        nc.vector.memset(cT, 0.0)

        def lstm_pointwise(work, g_ps, write_hall=None, iv=None):
            """gates PSUM [128, 16, 32] -> update hT, cT.

            Host folded 0.5 into i,f,o weight rows, so sigmoid(z) =
            (1 + tanh(z')) / 2 with z' the PSUM value; g uses tanh(z')
            directly (its rows unscaled). One tanh over all 16 chunks.
            """
            t_all = work.tile([128, GC, BL], b16, tag="tall")
            nc.scalar.activation(t_all, g_ps, AFT.Tanh, scale=1.0)
            sgi = work.tile([128, HC, BL], b16, tag="sgi")
            sgf = work.tile([128, HC, BL], b16, tag="sgf")
            sgo = work.tile([128, HC, BL], b16, tag="sgo")
            for sg, lo in ((sgf, 4), (sgi, 0)):
                nc.vector.tensor_scalar(out=sg, in0=t_all[:, lo:lo + 4, :],
                                        scalar1=1.0, scalar2=0.5,
                                        op0=ALU.add, op1=ALU.mult)
            u = work.tile([128, HC, BL], b16, tag="u")
            v = work.tile([128, HC, BL], b16, tag="v")
            nc.vector.tensor_mul(v, sgf, cT)
            nc.vector.tensor_mul(u, sgi, t_all[:, 8:12, :])
            nc.vector.tensor_add(cT, u, v)
            thc = work.tile([128, HC, BL], b16, tag="thc")
            nc.scalar.activation(thc, cT, AFT.Tanh, scale=1.0)
            nc.vector.tensor_scalar(out=sgo, in0=t_all[:, 12:16, :],
                                    scalar1=1.0, scalar2=0.5,
                                    op0=ALU.add, op1=ALU.mult)
            nc.vector.tensor_mul(hT, sgo, thc)
            if write_hall is not None:
                dst = write_hall[:, :, :, bass.ds(iv, 1)]
                nc.vector.tensor_copy(dst, hT.unsqueeze(-1))

        # ---------------- encoder ----------------
        with tc.tile_pool(name="encp", bufs=1) as encp, \
             tc.tile_pool(name="encw", bufs=3) as encw, \
             tc.tile_pool(name="encr", bufs=4) as encr, \
             tc.tile_pool(name="psg", bufs=2, space="PSUM") as psg, \
             tc.tile_pool(name="pse", bufs=2, space="PSUM") as pse:
            ewhT = encp.tile([128, HC, 4 * H], b16)
            ewb = encp.tile([2, 4 * H], b16)
            H_all = encp.tile([128, HC, BL, L], b16)
            nc.sync.dma_start(ewhT, ewhT_d)
            nc.sync.dma_start(ewb, ewb_d)

            def emit_eps(iv):
                # E_sb[iv] = w1 @ h_iv -- emitted one body later (hT still
                # holds h_iv there) so these mms fill the pointwise window
                e_ps = pse.tile([128, HC, BL], mybir.dt.float32, tag="eps")
                for pc in range(HC):
                    for kc in range(HC):
                        nc.tensor.matmul(
                            e_ps[:, pc, :],
                            lhsT=w1T[:, kc, pc * 128:(pc + 1) * 128],
                            rhs=hT[:, kc, :], start=(kc == 0), stop=(kc == 3))
                dst = E_sb[:, bass.ds(iv, 1), :]
                nc.vector.tensor_copy(
                    dst, e_ps.rearrange("p a b -> p (a b)").unsqueeze(1))

            def enc_body(iv, k):
                if k == 0:
                    dum = encr.tile([1, 1], b16, tag="dum")
                    nc.scalar.activation(dum, eye16[0:1, 0:1], AFT.Tanh,
                                         scale=1.0)
                xa_t = encr.tile([2, BL], b16, tag="xa")
                nc.vector.tensor_copy(xa_t, xa_sb[:, bass.ds(iv * BL, BL)])
                g_ps = psg.tile([128, GC, BL], mybir.dt.float32, tag="gps")
                for gc in range(GC):
                    nc.tensor.matmul(
                        g_ps[:, gc, :], lhsT=ewb[:, gc * 128:(gc + 1) * 128],
                        rhs=xa_t, start=(gc == 0), stop=False)
                for gc in range(GC):
                    for kc in range(HC):
                        nc.tensor.matmul(
                            g_ps[:, gc, :],
                            lhsT=ewhT[:, kc, gc * 128:(gc + 1) * 128],
                            rhs=hT[:, kc, :], start=False,
                            stop=(gc == GC - 1 and kc == HC - 1))
                if k > 0:
                    emit_eps(iv - 1)
                lstm_pointwise(encw, g_ps, write_hall=H_all, iv=iv)

            with tc.For_i(0, L // 8, 1) as ordv:
                for k in range(8):
                    enc_body(ordv * 8 + k, k)
                emit_eps(ordv * 8 + 7)

            # H_all [hsub, hc, b, l] -> E0T [l, b, hc, hsub] via 128 PE transposes
            with tc.tile_pool(name="pst", bufs=2, space="PSUM") as pst, \
                 tc.tile_pool(name="trw", bufs=3) as trw:
                for hc in range(HC):
                    for b in range(BL):
                        tr_ps = pst.tile([128, 128], b16, tag="tr")
                        nc.tensor.transpose(tr_ps, H_all[:, hc, b, :], eye16)
                        if (hc * BL + b) % 2 == 0:
                            nc.vector.tensor_copy(E0T[:, b, hc, :], tr_ps)
                        else:
                            nc.scalar.copy(E0T[:, b, hc, :], tr_ps)

        # ---------------- decoder ----------------
        with tc.tile_pool(name="decp", bufs=1) as decp, \
             tc.tile_pool(name="decw", bufs=3) as decw, \
             tc.tile_pool(name="decx", bufs=2) as decx, \
             tc.tile_pool(name="decr", bufs=4) as decr, \
             tc.tile_pool(name="psq", bufs=1, space="PSUM") as psq, \
             tc.tile_pool(name="psz", bufs=2, space="PSUM") as psz, \
             tc.tile_pool(name="psa", bufs=1, space="PSUM") as psa, \
             tc.tile_pool(name="psc", bufs=1, space="PSUM") as psc, \
             tc.tile_pool(name="psg2", bufs=2, space="PSUM") as psg2:
            T_sb = decp.tile([128, L, 128], b16)

            def dec_body(iv, k, ordv):
                if k == 0:
                    # tiny dummy: absorbs the per-iteration ACT_TABLE_LOAD
                    # while PE/DVE run the front phase (else it delays tanh c0)
                    dum = decr.tile([1, 1], b16, tag="dum")
                    nc.scalar.activation(dum, eye16[0:1, 0:1], AFT.Tanh,
                                         scale=1.0)
                # q.T [hsub, hc, b] -- first on PE after h so the tanh
                # window opens as early as possible
                q_ps = psq.tile([128, HC, BL], mybir.dt.float32, tag="qps")
                for pc in range(HC):
                    for kc in range(HC):
                        nc.tensor.matmul(
                            q_ps[:, pc, :],
                            lhsT=w2T[:, kc, pc * 128:(pc + 1) * 128],
                            rhs=hT[:, kc, :], start=(kc == 0), stop=(kc == 3))
                qT = decw.tile([128, HC, BL], b16, tag="qT")
                nc.vector.tensor_copy(qT, q_ps)
                qflat = qT.rearrange("p a b -> p (a b)")
                # gates: bias + Wh@h mms next -- they run on PE during the
                # ACT-bound tanh window. Wi@ctx appends after ctx below.
                dt_t = decr.tile([2, BL], b16, tag="dt")
                nc.vector.tensor_copy(dt_t, dt_sb[:, bass.ds(iv * BL, BL)])
                g_ps = psg2.tile([128, GC, BL], mybir.dt.float32, tag="gps2")
                for gc in range(GC):
                    nc.tensor.matmul(
                        g_ps[:, gc, :], lhsT=dwb[:, gc * 128:(gc + 1) * 128],
                        rhs=dt_t, start=(gc == 0), stop=False)
                for gc in range(GC):
                    for kc in range(HC):
                        nc.tensor.matmul(
                            g_ps[:, gc, :],
                            lhsT=dwhT[:, kc, gc * 128:(gc + 1) * 128],
                            rhs=hT[:, kc, :], start=False, stop=False)
                # X = E + q (broadcast over l), T = tanh(X); leading
                # blocks smaller so the first tanh can start sooner
                for lo, ln_ in ((0, 16), (16, 32), (48, 32), (80, 32),
                                (112, 16)):
                    X_blk = decx.tile([128, ln_, 128], b16, tag=f"X{ln_}")
                    q_b = bass.AP(tensor=qflat.tensor, offset=qflat.offset,
                                  ap=[qflat.ap[0], [0, ln_], qflat.ap[1]])
                    nc.vector.tensor_add(
                        X_blk, E_sb[:, lo:lo + ln_, :], q_b)
                    nc.scalar.activation(
                        T_sb[:, lo:lo + ln_, :], X_blk, AFT.Tanh, scale=1.0)
                # scores: Z[l] = T_l.T @ v4, extracted per l-block so the
                # DVE reduction hides under the tanh window
                Z_ps = psz.tile([128, L, HC], mybir.dt.float32, tag="zps")
                S_sb = decw.tile([BL, L], mybir.dt.float32, tag="S")
                for lo, ln_ in ((0, 16), (16, 32), (48, 32), (80, 32),
                                (112, 16)):
                    for l in range(lo, lo + ln_):
                        nc.tensor.matmul(Z_ps[:, l, :], lhsT=T_sb[:, l, :],
                                         rhs=v4, start=True, stop=True)
                    sl = slice(lo, lo + ln_)
                    nc.vector.tensor_copy(S_sb[:, sl], Z_ps[0:32, sl, 0])
                    nc.vector.tensor_add(S_sb[:, sl], S_sb[:, sl],
                                         Z_ps[32:64, sl, 1])
                    nc.vector.tensor_add(S_sb[:, sl], S_sb[:, sl],
                                         Z_ps[64:96, sl, 2])
                    nc.vector.tensor_add(S_sb[:, sl], S_sb[:, sl],
                                         Z_ps[96:128, sl, 3])
                # softmax without max-subtraction (|S| <~ 20, fp32-safe)
                e_sb = decw.tile([BL, L], mybir.dt.float32, tag="e")
                s_t = decw.tile([BL, 1], mybir.dt.float32, tag="s")
                nc.scalar.activation(e_sb, S_sb, AFT.Exp, scale=1.0)
                nc.vector.tensor_reduce(out=s_t, in_=e_sb,
                                        axis=mybir.AxisListType.X, op=ALU.add)
                # stash raw scores (off critical path) + per-step sums
                nc.sync.dma_start(sstore_d[bass.ds(iv, 1), :, :], S_sb)
                nc.vector.tensor_copy(s_all[:, bass.ds(iv, 1)], s_t)
                r = decw.tile([BL, 1], mybir.dt.float32, tag="r")
                nc.vector.reciprocal(r, s_t)
                a_sb = decw.tile([BL, L], b16, tag="a")
                nc.vector.tensor_scalar_mul(a_sb, e_sb, r)
                # context
                aT_ps = psa.tile([128, BL], b16, tag="aT")
                nc.tensor.transpose(aT_ps, a_sb, eye16[0:BL, 0:BL])
                aT = decw.tile([128, BL], b16, tag="aTs")
                nc.vector.tensor_copy(aT, aT_ps)
                ctx_ps = psc.tile([128, HC, BL], mybir.dt.float32, tag="cps")
                xT = decw.tile([128, HC, BL], b16, tag="xT")
                for hc in range(HC):
                    for b in range(BL):
                        nc.tensor.matmul(ctx_ps[:, hc, b:b + 1],
                                         lhsT=E0T[:, b, hc, :],
                                         rhs=aT[:, b:b + 1],
                                         start=True, stop=True)
                    # per-hc cast overlaps the next hc's ctx matmuls
                    nc.vector.tensor_copy(xT[:, hc, :], ctx_ps[:, hc, :])
                # gates: Wi@ctx completes the accumulation group (kc-outer so
                # each kc chunk only needs its own xT slice)
                for kc in range(HC):
                    for gc in range(GC):
                        nc.tensor.matmul(
                            g_ps[:, gc, :],
                            lhsT=dwiT[:, kc, gc * 128:(gc + 1) * 128],
                            rhs=xT[:, kc, :], start=False,
                            stop=(kc == HC - 1 and gc == GC - 1))
                lstm_pointwise(decw, g_ps)

            with tc.For_i(0, L // 8, 1) as ordv:
                for k in range(8):
                    dec_body(ordv * 8 + k, k, ordv)

        # ---------------- deferred log-softmax ----------------
        with tc.tile_pool(name="post", bufs=8) as post, \
             tc.tile_pool(name="postc", bufs=1) as postc:
            lns = postc.tile([BL, L], mybir.dt.float32)
            nc.scalar.activation(lns, s_all, AFT.Ln, scale=1.0)
            for t in range(L):
                S_t = post.tile([BL, L], mybir.dt.float32, tag="St")
                nc.scalar.dma_start(S_t, sstore_d[t, :, :])
                o_t = post.tile([BL, L], mybir.dt.float32, tag="ot")
                nc.vector.tensor_scalar(out=o_t, in0=S_t,
                                        scalar1=lns[:, t:t + 1], scalar2=None,
                                        op0=ALU.subtract)
                (nc.sync if t % 2 == 0 else nc.gpsimd).dma_start(
                    outp_d[:, t, :], o_t)

    nc.finalize()
    return nc


def _prep_weights(enc_Wi, enc_Wh, enc_b, dec_Wi, dec_Wh, dec_b, w1, w2, vt):
    """Host-side weight repack (shared across cores).

    0.5 folded into i,f,o gate rows so sigmoid(z) = (1+tanh(z'))/2 on-device.
    """
    f = np.float32
    sc = np.ones(4 * H, f)
    sc[0:H] = 0.5          # i
    sc[H:2 * H] = 0.5      # f
    sc[3 * H:4 * H] = 0.5  # o

    def chunkT(W):  # [4H, H] -> [128, HC, 4H]: out[p, kc, g] = W[g, kc*128+p]
        Wt = np.ascontiguousarray(W.astype(f).T)          # [H, 4H]
        return Wt.reshape(HC, 128, 4 * H).transpose(1, 0, 2).astype(bf16)

    def chunkT_sq(W):  # [H, H] -> [128, HC, H]
        Wt = np.ascontiguousarray(W.astype(f).T)
        return Wt.reshape(HC, 128, H).transpose(1, 0, 2).astype(bf16)

    ewb = np.stack([enc_Wi.astype(f)[:, 0] * sc, enc_b.astype(f) * sc]).astype(bf16)
    dwb = np.stack([dec_Wi.astype(f)[:, H] * sc, dec_b.astype(f) * sc]).astype(bf16)
    return {
        "ewb": ewb, "dwb": dwb,
        "ewhT": chunkT(enc_Wh * sc[:, None]),
        "dwhT": chunkT(dec_Wh * sc[:, None]),
        "dwiT": chunkT(dec_Wi[:, :H] * sc[:, None]),
        "w1T": chunkT_sq(w1), "w2T": chunkT_sq(w2),
        "v4": vt.astype(f)[0].reshape(HC, 128).T.astype(bf16).copy(),
    }


def kernel(xs, x_lens, argsort_xs, enc_Wi, enc_Wh, enc_b,
           dec_Wi, dec_Wh, dec_b, w1, w2, vt):
    from concourse.bass_utils import run_bass_kernel_spmd

    if "nc" not in _cache:
        _cache["nc"] = _build_nc()
    nc = _cache["nc"]

    wmap = _prep_weights(enc_Wi, enc_Wh, enc_b, dec_Wi, dec_Wh, dec_b,
                         w1, w2, vt)
    xs_f = xs.astype(np.float32)
    D = np.concatenate(
        [np.zeros((B, 1), np.float32),
         np.take_along_axis(xs_f, argsort_xs[:, :-1].astype(np.int64), axis=1)],
        axis=1)  # [B, L] teacher-forced decoder inputs

    in_maps = []
    for c in range(NCORES):
        sl = slice(c * BL, (c + 1) * BL)
        xa = np.empty((2, L * BL), np.float32)
        xa[0] = xs_f[sl].T.reshape(-1)       # xa[0, l*BL+b] = xs[b, l]
        xa[1] = 1.0
        dt = np.empty((2, L * BL), np.float32)
        dt[0] = D[sl].T.reshape(-1)
        dt[1] = 1.0
        m = dict(wmap)
        m["xa"] = xa.astype(bf16)
        m["dt"] = dt.astype(bf16)
        in_maps.append(m)

    _cache["in_maps"] = in_maps
    res = run_bass_kernel_spmd(nc, in_maps, core_ids=list(range(NCORES)))
    out = np.concatenate([res.results[c]["outp"] for c in range(NCORES)], axis=0)
    return np.ascontiguousarray(out.astype(np.float32))
